# revision 33
# baseline (speedup 1.0000x reference)
"""CLIP attention (B=2, S=2048, H=768, 12 heads) on 8 trn2 NeuronCores.

Sharding: data-parallel over batch (2) x tensor-parallel over head groups
(4 groups of 3 heads).  Each core computes, for its (batch, head-group):
    q = x @ Wq_g * 1/sqrt(64) (+ bq_g scaled)      [2048, 192]
    k = x @ Wk_g                                    [2048, 192]
    v = x @ Wv_g                                    [2048, 192]
    per head: P' = exp(q k^T)   (no max subtraction; logits are O(1))
              O'^T, s via ones-augmented V:  o_ps = [V_h | 1]^T @ P'^T
    y = sum_h (O'_h / s_h) @ Wo_h                   [2048, 768]
Host sums the 4 head-group partials per batch (bf16 partials, f32 sum) and
adds the exactly-folded bias terms (bk drops out of softmax; bv/bo fold).

Matmul convention: nc.tensor.matmul(out, lhsT, rhs) => out = lhsT.T @ rhs,
contraction over the partition dim of both operands.

v8 (from v3, ~179us -> ~159us):
  - prologue: weights host-packed contiguous; x DMA split by TOKEN range
    (a1=tokens 0:512 first) with add_dep chaining so the critical set
    (wqk + a1) gets full HBM bandwidth -> first exp ~18us (was ~27).
  - 16 dummy matmuls on zeros warm the PE p-state to 2.4GHz during the
    DMA wait (cold matmuls run at ~1.2GHz).
  - dummy exp preloads the ACT table off the critical path.
  - attention: ONE flat software-pipelined stream over (pass, kb) steps;
    the logits pair of step s+1 is emitted before the oV matmuls of
    step s and carries tc.high_priority() so the Tile scheduler keeps
    the row-group pair adjacent (they co-execute on disjoint PE rows)
    and never queues drip matmuls ahead of them.
  - v0/v1 run as pre-drips of step (0,0) (NOT phase A) so the attention
    PSUM pools (ltp 4 + opp 2 + flex 2 banks) open immediately.
  - pass order [p01(0), p01(1), h2(0), p01(2), h2(1), p01(3)] with Y for
    token blocks 8-11 dripped into the last pass; blocks 12-15 get their
    oTs2 halves pre-started during the final softmax-scale chain.
  - extract copies are high-priority (they gate the o-psum slots the
    next pass's first oV needs); last pass sends the oraw copies to the
    idle scalar engine.
  - output bf16 (halves output DMA; host sums partials in f32).
"""

import sys

if "/opt/trn_rl_repo" not in sys.path:
    sys.path.insert(0, "/opt/trn_rl_repo")

import numpy as np
import ml_dtypes

import concourse.bacc as bacc
import concourse.tile as tile
from concourse import mybir
from concourse.bass_utils import run_bass_kernel_spmd
from concourse.tile import add_dep_helper

BF16 = mybir.dt.bfloat16
F32 = mybir.dt.float32
MULT = mybir.AluOpType.mult

S = 2048          # sequence length
C = 768           # hidden
HD = 64           # head dim
NCORES = 8
GROUPS = 4        # head groups (tensor parallel)
HPG = 3           # heads per group
GF = HPG * HD     # group feature width = 192
NCC = C // 128    # contraction chunks = 6
NQB = S // 128    # token blocks = 16
NKB = S // 128    # key blocks = 16
QH = S // 2       # queries per half = 1024


def build_program():
    nc = bacc.Bacc("TRN2", target_bir_lowering=False, debug=False)

    xt_dram = nc.dram_tensor("x", (C, S), BF16, kind="ExternalInput").ap()
    xt_pcs = xt_dram.rearrange("(c p) s -> p c s", p=128)
    wqk = nc.dram_tensor("wqk", (128, NCC, 256), BF16, kind="ExternalInput").ap()
    wqk2 = nc.dram_tensor("wqk2", (128, NCC, 128), BF16, kind="ExternalInput").ap()
    wv = nc.dram_tensor("wv", (128, NCC, GF), BF16, kind="ExternalInput").ap()
    wo = nc.dram_tensor("wo", (128, 2, C), BF16, kind="ExternalInput").ap()
    bqc = nc.dram_tensor("bqc", (128, 2), F32, kind="ExternalInput").ap()
    out = nc.dram_tensor("out", (S, C), BF16, kind="ExternalOutput").ap()

    with tile.TileContext(nc) as tc:
        with tc.tile_pool(name="consts", bufs=1) as consts:
            wqk_sb = consts.tile([128, NCC, 256], BF16)
            wqk2_sb = consts.tile([128, NCC, 128], BF16)
            wv_sb = consts.tile([128, NCC, GF], BF16)
            wo_sb = consts.tile([128, 2, C], BF16)
            bq_sb = consts.tile([128, 2], F32)
            xT_sb = consts.tile([128, NCC, S], BF16)
            xT = [xT_sb[:, c, :] for c in range(NCC)]
            wq_c = [wqk_sb[:, c, 0:128] for c in range(NCC)]
            wk_c = [wqk_sb[:, c, 128:256] for c in range(NCC)]
            wqk2_c = [wqk2_sb[:, c, :] for c in range(NCC)]
            wo01_sb = wo_sb[:, 0, :]
            wo2_sb = wo_sb[:, 1, :]
            # dummy 1-elem exp: forces the ACT table load during the DMA
            # wait instead of on the first real exp (saves ~1.3us)
            dummy = consts.tile([1, 2], F32, name="dummy")
            nc.vector.memset(dummy[:], 0.0)
            nc.scalar.activation(dummy[:], dummy[:],
                                 mybir.ActivationFunctionType.Exp)
            # PE warm-up fodder (see pp pool below): zeros tile
            wu = consts.tile([128, 512], BF16, name="wu")
            nc.vector.memset(wu[:], 0.0)
            # prologue: 3 DMA rings; x split by TOKEN range so attention can
            # start after only tokens 0:512 (a1a+a1b) land.  Later x ranges
            # are chained BEHIND a1 (add_dep) so the critical prologue set
            # (wqk + a1) gets the full HBM bandwidth.
            #   scalar(HWDGE): a1a = x[c0:c3, 0:512],  a3 = x[:, 1024:1536]
            #   sync  (HWDGE): wqk, a1b = x[c3:c6, 0:512], a2, a4
            #   gpsimd(SWDGE): bqc, wv, wqk2, wo
            h_a1a = nc.scalar.dma_start(out=xT_sb[:, 0:3, 0:512],
                                        in_=xt_pcs[:, 0:3, 0:512])
            nc.sync.dma_start(out=wqk_sb[:], in_=wqk[:])
            nc.gpsimd.dma_start(out=bq_sb[:], in_=bqc[:])
            h_a1b = nc.sync.dma_start(out=xT_sb[:, 3:6, 0:512],
                                      in_=xt_pcs[:, 3:6, 0:512])
            nc.gpsimd.dma_start(out=wv_sb[:], in_=wv[:])
            h_a2 = nc.sync.dma_start(out=xT_sb[:, :, 512:1024],
                                     in_=xt_pcs[:, :, 512:1024])
            h_a3 = nc.scalar.dma_start(out=xT_sb[:, :, 1024:1536],
                                       in_=xt_pcs[:, :, 1024:1536])
            nc.sync.dma_start(out=xT_sb[:, :, 1536:2048],
                              in_=xt_pcs[:, :, 1536:2048])
            h_wqk2 = nc.gpsimd.dma_start(out=wqk2_sb[:], in_=wqk2[:])
            nc.gpsimd.dma_start(out=wo_sb[:], in_=wo[:])
            add_dep_helper(h_a2.ins, h_a1a.ins,
                           reason="x range 2 waits for critical range 1")
            add_dep_helper(h_a3.ins, h_a1b.ins,
                           reason="x range 3 waits for critical range 1")
            add_dep_helper(h_wqk2.ins, h_a2.ins,
                           reason="wqk2 not needed until pass 1")

            # heads 0,1 combined (h0 rows 0-63, h1 rows 64-127)
            qT01 = consts.tile([128, S], BF16, name="qT01")
            kT01 = consts.tile([128, S], BF16, name="kT01")
            # head 2, rows duplicated for row-group pairing
            qT2 = consts.tile([128, S], BF16, name="qT2")
            kT2 = consts.tile([128, S], BF16, name="kT2")
            vS = [consts.tile([128, HPG, HD + 1], BF16, name=f"vS{t}")
                  for t in range(NKB)]
            # PRE-SCALED attention outputs (h0 rows 0-63, h1 rows 64-127)
            oTs01 = [consts.tile([128, QH], BF16, name=f"oTs01_{qh}")
                     for qh in range(2)]
            oTs2 = [consts.tile([128, QH], BF16, name=f"oTs2_{qh}")
                    for qh in range(2)]
            s_bf = [consts.tile([1, S], F32, name=f"s_bf{h}")
                    for h in range(HPG)]
            # bf16 DMA staging for all 16 blocks
            ysb = [consts.tile([128, C], BF16, name=f"ysb{t}")
                   for t in range(NQB)]

            def qk_piece_body(pool, tag, w_list, n, kind):
                ps = pool.tile([128, 512], F32, tag=tag, name=f"{kind}{n}")
                for c in range(NCC):
                    nc.tensor.matmul(
                        ps[:],
                        w_list[c],
                        xT[c][:, n * 512 : (n + 1) * 512],
                        start=(c == 0),
                        stop=(c == NCC - 1),
                    )
                ns = slice(n * 512, (n + 1) * 512)
                if kind == "q":      # one combined copy + bias
                    nc.vector.tensor_scalar_add(
                        qT01[:, ns], ps[:], bq_sb[:, 0:1]
                    )
                elif kind == "k":
                    nc.vector.tensor_copy(kT01[:, ns], ps[:])
                else:  # 'qk2': q2 rows 0-63 (+bias, dup), k2 rows 64-127 (dup)
                    nc.vector.tensor_scalar_add(
                        qT2[0:64, ns], ps[0:64, :], bq_sb[0:64, 1:2]
                    )
                    nc.vector.tensor_scalar_add(
                        qT2[64:128, ns], ps[0:64, :], bq_sb[64:128, 1:2]
                    )
                    nc.vector.tensor_copy(kT2[0:64, ns], ps[64:128, :])
                    nc.vector.tensor_copy(kT2[64:128, ns], ps[64:128, :])

            def v_piece_body(pool, tag, t):
                vps = pool.tile([128, GF], F32, tag=tag, name=f"vp{t}")
                for c in range(NCC):
                    nc.tensor.matmul(
                        vps[:],
                        xT[c][:, t * 128 : (t + 1) * 128],
                        wv_sb[:, c, :],
                        start=(c == 0),
                        stop=(c == NCC - 1),
                    )
                nc.vector.tensor_copy(
                    vS[t][:, :, 0:HD],
                    vps[:].rearrange("p (h d) -> p h d", h=HPG),
                )
                nc.vector.memset(vS[t][:, :, HD : HD + 1], 1.0)

            # ------------- phase A: just enough to start attention -------------
            # v0/v1 are NOT computed here: they run as pre-drips of step
            # (0,0) so phase A only holds 2 PSUM banks and the attention
            # pools (ltp 4 + opp 2 + flex 2) can open without waiting on
            # serialized DVE copies.
            with tc.tile_pool(name="pp", bufs=2, space="PSUM") as pp:
                # q0/k0 interleaved by contraction chunk: the PE pipelines
                # across the xT chunk arrivals instead of stalling per piece
                ps_q = pp.tile([128, 512], F32, tag="pp", name="q0")
                ps_k = pp.tile([128, 512], F32, tag="pp", name="k0")
                # PE warm-up: dummy matmuls on zeros into ps_q (overwritten
                # by q0's start=True) while the x DMA is in flight, so the
                # PE p-state is at full clock when phase A starts
                for _ in range(16):
                    nc.tensor.matmul(ps_q[:], wu[:, 0:128], wu[:],
                                     start=True, stop=True)
                for c in range(NCC):
                    nc.tensor.matmul(ps_q[:], wq_c[c], xT[c][:, 0:512],
                                     start=(c == 0), stop=(c == NCC - 1))
                    nc.tensor.matmul(ps_k[:], wk_c[c], xT[c][:, 0:512],
                                     start=(c == 0), stop=(c == NCC - 1))
                nc.vector.tensor_scalar_add(qT01[:, 0:512], ps_q[:],
                                            bq_sb[:, 0:1])
                nc.vector.tensor_copy(kT01[:, 0:512], ps_k[:])

            # ---------------- attention: one pipelined stream ----------------
            with tc.tile_pool(name="flex", bufs=1, space="PSUM") as flex, \
                 tc.tile_pool(name="lt_ps", bufs=2, space="PSUM") as ltp, \
                 tc.tile_pool(name="o_ps", bufs=2, space="PSUM") as opp, \
                 tc.tile_pool(name="att_sb", bufs=4) as asb:

                def task_v(t):
                    def run():
                        v_piece_body(flex, "flex", t)
                    return run

                def task_qk(w_list, n, kind):
                    def run():
                        qk_piece_body(flex, "flex", w_list, n, kind)
                    return run

                ext_oraw = {}

                # deferred part of the softmax-scale chain (dripped into the
                # NEXT pass so the broadcast never stalls the in-order queues)
                def ext_finish(h, qa):
                    def run():
                        sb = asb.tile([HD, 512], F32, tag="sbc",
                                      name=f"sb{h}_{qa}")
                        nc.gpsimd.partition_broadcast(
                            sb[:], s_bf[h][:, qa : qa + 512], channels=HD
                        )
                        rb = asb.tile([HD, 512], F32, tag="rb",
                                      name=f"rb{h}_{qa}")
                        nc.vector.reciprocal_approx_fast(out=rb[:], in_=sb[:])
                        oraw = ext_oraw.pop((h, qa))
                        cs = slice(qa % QH, (qa % QH) + 512)
                        if h < 2:
                            dsts = [oTs01[qa // QH][h * 64 : (h + 1) * 64, cs]]
                        else:
                            dsts = [oTs2[qa // QH][0:64, cs],
                                    oTs2[qa // QH][64:128, cs]]
                        for dst in dsts:
                            nc.vector.tensor_tensor(out=dst, in0=oraw[:],
                                                    in1=rb[:], op=MULT)
                    return run

                def extract(o_ps, h, qa):
                    """Immediate extraction: frees o_ps (s row + raw O').
                    High priority: these copies gate the o-psum slots the
                    next pass's first oV matmuls need."""
                    with tc.high_priority():
                        nc.vector.tensor_copy(s_bf[h][:, qa : qa + 512],
                                              o_ps[HD : HD + 1, :])
                        oraw = asb.tile([HD, 512], BF16, tag="oraw",
                                        name=f"oraw{h}_{qa}")
                        nc.vector.tensor_copy(oraw[:], o_ps[0:HD, :])
                    ext_oraw[(h, qa)] = oraw

                def task_y_full(qh, t):
                    """single-shot Y for a token block: all 3 heads."""
                    def run():
                        yp = flex.tile([128, C], F32, tag="flex",
                                       name=f"y{qh}_{t}")
                        tbs = slice((t % 8) * 128, (t % 8) * 128 + 128)
                        nc.tensor.matmul(yp[:, 0:512], oTs01[qh][:, tbs],
                                         wo01_sb[:, 0:512],
                                         start=True, stop=False)
                        nc.tensor.matmul(yp[:, 512:C], oTs01[qh][:, tbs],
                                         wo01_sb[:, 512:C],
                                         start=True, stop=False)
                        nc.tensor.matmul(yp[:, 0:512], oTs2[qh][0:64, tbs],
                                         wo2_sb[0:64, 0:512],
                                         start=False, stop=True)
                        nc.tensor.matmul(yp[:, 512:C], oTs2[qh][64:128, tbs],
                                         wo2_sb[64:128, 512:C],
                                         start=False, stop=True)
                        nc.vector.tensor_copy(ysb[t][:], yp[:])
                        # sync queue only: the scalar engine is busy with exp
                        nc.sync.dma_start(out=out[t * 128 : (t + 1) * 128, :],
                                          in_=ysb[t][:])
                    return run

                # pass specs ------------------------------------------------
                def p01(qb):
                    qa = qb * 512
                    return (kT01, qT01, qa, kT01, qT01, qa,
                            (0, qa), (1, qa))

                def h2(qh):
                    qa = qh * QH
                    return (kT2, qT2, qa, kT2, qT2, qa + 512,
                            (2, qa), (2, qa + 512))

                passes = [p01(0), p01(1), h2(0), p01(2), h2(1), p01(3)]
                extra = {
                    0: [(0, task_v(2)), (0, task_v(3)), (1, task_v(4)),
                        (2, task_qk(wk_c, 1, "k")), (3, task_v(5)),
                        (4, task_v(6)), (5, task_qk(wk_c, 2, "k")),
                        (5, task_v(7)), (6, task_v(8)), (7, task_v(9)),
                        (8, task_qk(wk_c, 3, "k")), (8, task_v(10)),
                        (9, task_v(11)), (10, task_v(12)), (11, task_v(13)),
                        (12, task_v(14)), (13, task_v(15)),
                        (14, task_qk(wq_c, 1, "q"))],
                    # qk2 pieces 0+1 EARLY: h2(0)'s very first logits
                    # pair reads qT2 piece 0 (rows 0:64) AND piece 1 (rows
                    # 64:128) — both must clear the DVE dup copies well
                    # before the pass-1 -> pass-2 boundary
                    1: [(2, task_qk(wqk2_c, 0, "qk2")),
                        (5, task_qk(wqk2_c, 1, "qk2")),
                        (8, task_qk(wq_c, 2, "q"))],
                    2: [(2, task_qk(wqk2_c, 2, "qk2")),
                        (4, task_qk(wq_c, 3, "q")),
                        (6, task_qk(wqk2_c, 3, "qk2"))],
                    # p01(2): first-half Y blocks 0-3 (kb 0 kept drip-free:
                    # transitions are the tightest PE windows)
                    3: [(1, task_y_full(0, 0)), (4, task_y_full(0, 1)),
                        (7, task_y_full(0, 2)), (10, task_y_full(0, 3))],
                    # h2(1): first-half Y blocks 4-7
                    4: [(1, task_y_full(0, 4)), (4, task_y_full(0, 5)),
                        (7, task_y_full(0, 6)), (10, task_y_full(0, 7))],
                    # p01(3): blocks 8-11 single-shot, early so the pass's
                    # final kbs are drip-free ahead of the tail chain
                    5: [(1, task_y_full(1, 8)), (4, task_y_full(1, 9)),
                        (7, task_y_full(1, 10)), (10, task_y_full(1, 11))],
                }
                drips = {}
                for i, lst in extra.items():
                    for kb, ts in lst:
                        drips.setdefault((i, kb), []).append(ts)

                steps = [(i, kb) for i in range(len(passes))
                         for kb in range(NKB)]

                def emit_lt(i, kb):
                    (klo, qlo_t, qlo, khi, qhi_t, qhi, uA, uB) = passes[i]
                    kbs = slice(kb * 128, (kb + 1) * 128)
                    lt = ltp.tile([128, 1024], F32, tag="lt")
                    # high priority: the scheduler must keep the row-group
                    # pair adjacent (they co-execute on disjoint PE rows)
                    # and never insert drip matmuls before them — exp(s+1)
                    # is gated on this pair.  Priorities must be UNIQUE and
                    # ordered per step: with a constant (0,1) per pair, two
                    # simultaneously-ready pairs pop as (h0,h0,h64,h64) —
                    # same-row-group halves serialize on the PE.
                    po = tc.cur_priority
                    tc.cur_priority = 2 * (i * NKB + kb)
                    nc.tensor.matmul(
                        lt[:, 0:512], klo[0:64, kbs],
                        qlo_t[0:64, qlo : qlo + 512],
                        start=True, stop=True,
                    )
                    nc.tensor.matmul(
                        lt[:, 512:1024], khi[64:128, kbs],
                        qhi_t[64:128, qhi : qhi + 512],
                        start=True, stop=True,
                    )
                    tc.cur_priority = po
                    return lt

                o_cur = {}
                lt_cur = emit_lt(*steps[0])
                for s, (i, kb) in enumerate(steps):
                    uA, uB = passes[i][6], passes[i][7]
                    elt = asb.tile([128, 1024], BF16, tag="elt")
                    nc.scalar.activation(
                        elt[:], lt_cur[:], mybir.ActivationFunctionType.Exp
                    )
                    # software pipeline: next step's logits before this
                    # step's oV matmuls (keeps Act back-to-back)
                    if s + 1 < len(steps):
                        lt_cur = emit_lt(*steps[s + 1])
                    if s == 0:
                        # v0/v1 must be on the PE queue BEFORE the first oV
                        # matmuls (in-order queue: oV(0,0) waits on vS[0])
                        v_piece_body(flex, "flex", 0)
                        v_piece_body(flex, "flex", 1)
                    if kb == 0:
                        o_cur[0] = opp.tile([HD + 1, 512], F32, tag="o",
                                            name=f"oA{uA[0]}_{uA[1]}")
                        o_cur[1] = opp.tile([HD + 1, 512], F32, tag="o",
                                            name=f"oB{uB[0]}_{uB[1]}")
                    nc.tensor.matmul(
                        o_cur[0][:], vS[kb][:, uA[0], :], elt[:, 0:512],
                        start=(kb == 0), stop=(kb == NKB - 1),
                    )
                    nc.tensor.matmul(
                        o_cur[1][:], vS[kb][:, uB[0], :], elt[:, 512:1024],
                        start=(kb == 0), stop=(kb == NKB - 1),
                    )
                    for tsk in drips.get((i, kb), ()):
                        tsk()
                    if kb == NKB - 1:
                        last = i == len(passes) - 1
                        if not last:
                            extract(o_cur[0], *uA)
                            extract(o_cur[1], *uB)
                        else:
                            # last pass: s rows via DVE (hp), oraw copies
                            # via the now-idle scalar engine
                            for oc, u in ((o_cur[0], uA), (o_cur[1], uB)):
                                with tc.high_priority():
                                    nc.vector.tensor_copy(
                                        s_bf[u[0]][:, u[1] : u[1] + 512],
                                        oc[HD : HD + 1, :])
                                    orw = asb.tile([HD, 512], BF16,
                                                   tag="oraw",
                                                   name=f"orw{u[0]}_{u[1]}")
                                    nc.scalar.copy(orw[:], oc[0:HD, :])
                                ext_oraw[u] = orw
                        if last:
                            # pre-start the oTs2 half of tail-Y blocks
                            # 12-14 NOW: keeps the PE busy (and its p-state
                            # up) during the final softmax-scale chain
                            yps_tail = {}
                            for t, (pool_t, tg) in zip(
                                (12, 13, 14),
                                ((flex, "flex"), (ltp, "lt"), (ltp, "lt")),
                            ):
                                tbs = slice((t - 8) * 128,
                                            (t - 8) * 128 + 128)
                                yp = pool_t.tile([128, C], F32, tag=tg,
                                                 name=f"yt_{t}")
                                nc.tensor.matmul(
                                    yp[:, 0:512], oTs2[1][0:64, tbs],
                                    wo2_sb[0:64, 0:512],
                                    start=True, stop=False)
                                nc.tensor.matmul(
                                    yp[:, 512:C], oTs2[1][64:128, tbs],
                                    wo2_sb[64:128, 512:C],
                                    start=True, stop=False)
                                yps_tail[t] = yp
                            # chunked scale chain: 4x128-col mult pieces
                            # per unit, interleaved A/B by token block, so
                            # each tail-Y block starts as soon as ITS
                            # columns of oTs01 are scaled instead of
                            # waiting for the full 512-col mults
                            qa = uA[1]
                            rbs = []
                            for j, h in enumerate((uA[0], uB[0])):
                                sbx = asb.tile([HD, 512], F32, tag="sbc",
                                               name=f"sbx{j}")
                                nc.gpsimd.partition_broadcast(
                                    sbx[:], s_bf[h][:, qa : qa + 512],
                                    channels=HD)
                                rbx = asb.tile([HD, 512], F32, tag="rb",
                                               name=f"rbx{j}")
                                nc.vector.reciprocal_approx_fast(
                                    out=rbx[:], in_=sbx[:])
                                rbs.append(rbx)
                            cs0 = qa % QH
                            for b in range(4):
                                bs = slice(b * 128, (b + 1) * 128)
                                ob = slice(cs0 + b * 128,
                                           cs0 + (b + 1) * 128)
                                for j, u in enumerate((uA, uB)):
                                    h = u[0]
                                    nc.vector.tensor_tensor(
                                        out=oTs01[qa // QH][
                                            h * 64 : (h + 1) * 64, ob],
                                        in0=ext_oraw[u][:, bs],
                                        in1=rbs[j][:, bs], op=MULT)
                            ext_oraw.pop(uA)
                            ext_oraw.pop(uB)
                        else:
                            ext_finish(*uA)()
                            ext_finish(*uB)()

                # tail: blocks 12-15 (need oTs01[1] 2nd half from last pass)
                for t in range(12, 16):
                    tbs = slice((t - 8) * 128, (t - 8) * 128 + 128)
                    if t in yps_tail:
                        yp = yps_tail[t]
                        nc.tensor.matmul(yp[:, 0:512], oTs01[1][:, tbs],
                                         wo01_sb[:, 0:512],
                                         start=False, stop=True)
                        nc.tensor.matmul(yp[:, 512:C], oTs01[1][:, tbs],
                                         wo01_sb[:, 512:C],
                                         start=False, stop=True)
                    else:
                        yp = flex.tile([128, C], F32, tag="flex",
                                       name=f"yt_{t}")
                        nc.tensor.matmul(yp[:, 0:512], oTs01[1][:, tbs],
                                         wo01_sb[:, 0:512],
                                         start=True, stop=False)
                        nc.tensor.matmul(yp[:, 512:C], oTs01[1][:, tbs],
                                         wo01_sb[:, 512:C],
                                         start=True, stop=False)
                        nc.tensor.matmul(yp[:, 0:512], oTs2[1][0:64, tbs],
                                         wo2_sb[0:64, 0:512],
                                         start=False, stop=True)
                        nc.tensor.matmul(yp[:, 512:C], oTs2[1][64:128, tbs],
                                         wo2_sb[64:128, 512:C],
                                         start=False, stop=True)
                    # split the psum->bf16 casts across DVE and the (now
                    # idle) scalar engine so the tail doesn't serialize
                    if t % 2 == 0:
                        nc.scalar.copy(ysb[t][:], yp[:])
                    else:
                        nc.vector.tensor_copy(ysb[t][:], yp[:])
                    eng = nc.sync if t % 2 == 0 else nc.scalar
                    eng.dma_start(out=out[t * 128 : (t + 1) * 128, :],
                                  in_=ysb[t][:])

    nc.compile()
    return nc


_COMPILED_NC = None


def _get_nc():
    global _COMPILED_NC
    if _COMPILED_NC is None:
        _COMPILED_NC = build_program()
    return _COMPILED_NC


def _pack_chunks(w):
    # [768, F] -> [128, NCC, F]: partition p, chunk c <- row c*128+p
    f = w.shape[1]
    return np.ascontiguousarray(
        w.reshape(NCC, 128, f).transpose(1, 0, 2)
    )


def make_in_maps(x, Wq, bq, Wk, bk, Wv, bv, Wo, bo):
    scale = 1.0 / np.sqrt(HD)
    bf = ml_dtypes.bfloat16
    # host-side pre-transpose: kernel takes x^T [C, S]
    x_bf = [np.ascontiguousarray(x[b].T).astype(bf) for b in range(x.shape[0])]
    Wq = np.asarray(Wq)
    Wk = np.asarray(Wk)
    Wv = np.asarray(Wv)
    Wo = np.asarray(Wo)
    bq = np.asarray(bq)
    in_maps = []
    for cid in range(NCORES):
        b, g = divmod(cid, GROUPS)
        cols = slice(g * GF, (g + 1) * GF)
        wq_g = Wq[:, cols] * scale
        wk_g = Wk[:, cols]
        wqk2 = np.concatenate([wq_g[:, 128:192], wk_g[:, 128:192]], axis=1)
        wqk_h = np.concatenate([wq_g[:, 0:128], wk_g[:, 0:128]], axis=1)
        bq_g = bq[cols] * scale
        bqc = np.empty((128, 2), dtype=np.float32)
        bqc[:, 0] = bq_g[0:128]            # [bq_h0 | bq_h1]
        bqc[0:64, 1] = bq_g[128:192]       # bq_h2 duplicated
        bqc[64:128, 1] = bq_g[128:192]
        wo_g = Wo[cols, :]
        wo_h = np.stack(
            [wo_g[0:128, :],
             np.concatenate([wo_g[128:192, :]] * 2, axis=0)], axis=1
        )  # [128, 2, 768]
        in_maps.append(
            {
                "x": x_bf[b],
                "wqk": _pack_chunks(wqk_h).astype(bf),
                "wqk2": _pack_chunks(wqk2).astype(bf),
                "wv": _pack_chunks(Wv[:, cols]).astype(bf),
                "wo": np.ascontiguousarray(wo_h).astype(bf),
                "bqc": bqc,
            }
        )
    return in_maps


def gather_output(results, x, Wv, bv, Wo, bo):
    B = x.shape[0]
    out = np.zeros((B, S, C), dtype=np.float32)
    for cid in range(NCORES):
        b, _ = divmod(cid, GROUPS)
        out[b] += results[cid]["out"].astype(np.float32)
    # exact bias folds: bk cancels in softmax; v-bias -> bv @ Wo; + bo
    out += (np.asarray(bv, np.float32) @ np.asarray(Wo, np.float32)
            + np.asarray(bo, np.float32))
    return out


def kernel(x, Wq, bq, Wk, bk, Wv, bv, Wo, bo):
    x = np.asarray(x)
    nc = _get_nc()
    in_maps = make_in_maps(x, Wq, bq, Wk, bk, Wv, bv, Wo, bo)
    res = run_bass_kernel_spmd(nc, in_maps, core_ids=list(range(NCORES)))
    return gather_output(res.results, x, Wv, bv, Wo, bo)


# revision 34
# speedup vs baseline: 1.2166x; 1.2166x over previous
"""CLIP attention (B=2, S=2048, H=768, 12 heads) on 8 trn2 NeuronCores.

Sharding: data-parallel over batch (2) x tensor-parallel over head groups
(4 groups of 3 heads).  Each core computes, for its (batch, head-group):
    q = x @ Wq_g * 1/sqrt(64) (+ bq_g scaled)      [2048, 192]
    k = x @ Wk_g                                    [2048, 192]
    v = x @ Wv_g                                    [2048, 192]
    per head: P' = exp(q k^T)   (no max subtraction; logits are O(1))
              O'^T, s via ones-augmented V:  o_ps = [V_h | 1]^T @ P'^T
    y = sum_h (O'_h / s_h) @ Wo_h                   [2048, 768]
Host sums the 4 head-group partials per batch (bf16 partials, f32 sum) and
adds the exactly-folded bias terms (bk drops out of softmax; bv/bo fold).

Matmul convention: nc.tensor.matmul(out, lhsT, rhs) => out = lhsT.T @ rhs,
contraction over the partition dim of both operands.

v8 (from v3, ~179us -> ~159us):
  - prologue: weights host-packed contiguous; x DMA split by TOKEN range
    (a1=tokens 0:512 first) with add_dep chaining so the critical set
    (wqk + a1) gets full HBM bandwidth -> first exp ~18us (was ~27).
  - 16 dummy matmuls on zeros warm the PE p-state to 2.4GHz during the
    DMA wait (cold matmuls run at ~1.2GHz).
  - dummy exp preloads the ACT table off the critical path.
  - attention: ONE flat software-pipelined stream over (pass, kb) steps;
    the logits pair of step s+1 is emitted before the oV matmuls of
    step s and carries tc.high_priority() so the Tile scheduler keeps
    the row-group pair adjacent (they co-execute on disjoint PE rows)
    and never queues drip matmuls ahead of them.
  - v0/v1 run as pre-drips of step (0,0) (NOT phase A) so the attention
    PSUM pools (ltp 4 + opp 2 + flex 2 banks) open immediately.
  - pass order [p01(0), p01(1), h2(0), p01(2), h2(1), p01(3)] with Y for
    token blocks 8-11 dripped into the last pass; blocks 12-15 get their
    oTs2 halves pre-started during the final softmax-scale chain.
  - extract copies are high-priority (they gate the o-psum slots the
    next pass's first oV needs); last pass sends the oraw copies to the
    idle scalar engine.
  - output bf16 (halves output DMA; host sums partials in f32).
"""

import sys

if "/opt/trn_rl_repo" not in sys.path:
    sys.path.insert(0, "/opt/trn_rl_repo")

import numpy as np
import ml_dtypes

import concourse.bacc as bacc
import concourse.tile as tile
from concourse import mybir
from concourse.bass_utils import run_bass_kernel_spmd
from concourse.tile import add_dep_helper

BF16 = mybir.dt.bfloat16
F32 = mybir.dt.float32
MULT = mybir.AluOpType.mult

S = 2048          # sequence length
C = 768           # hidden
HD = 64           # head dim
NCORES = 8
GROUPS = 4        # head groups (tensor parallel)
HPG = 3           # heads per group
GF = HPG * HD     # group feature width = 192
NCC = C // 128    # contraction chunks = 6
NQB = S // 128    # token blocks = 16
NKB = S // 128    # key blocks = 16
QH = S // 2       # queries per half = 1024


def build_program():
    nc = bacc.Bacc("TRN2", target_bir_lowering=False, debug=False)

    xt_dram = nc.dram_tensor("x", (C, S), BF16, kind="ExternalInput").ap()
    xt_pcs = xt_dram.rearrange("(c p) s -> p c s", p=128)
    wqk = nc.dram_tensor("wqk", (128, NCC, 256), BF16, kind="ExternalInput").ap()
    wqk2 = nc.dram_tensor("wqk2", (128, NCC, 128), BF16, kind="ExternalInput").ap()
    wv = nc.dram_tensor("wv", (128, NCC, GF), BF16, kind="ExternalInput").ap()
    wo = nc.dram_tensor("wo", (128, 2, C), BF16, kind="ExternalInput").ap()
    bqc = nc.dram_tensor("bqc", (128, 2), F32, kind="ExternalInput").ap()
    out = nc.dram_tensor("out", (S, C), BF16, kind="ExternalOutput").ap()

    with tile.TileContext(nc) as tc:
        with tc.tile_pool(name="consts", bufs=1) as consts:
            wqk_sb = consts.tile([128, NCC, 256], BF16)
            wqk2_sb = consts.tile([128, NCC, 128], BF16)
            wv_sb = consts.tile([128, NCC, GF], BF16)
            wo_sb = consts.tile([128, 2, C], BF16)
            bq_sb = consts.tile([128, 2], F32)
            xT_sb = consts.tile([128, NCC, S], BF16)
            xT = [xT_sb[:, c, :] for c in range(NCC)]
            wq_c = [wqk_sb[:, c, 0:128] for c in range(NCC)]
            wk_c = [wqk_sb[:, c, 128:256] for c in range(NCC)]
            wqk2_c = [wqk2_sb[:, c, :] for c in range(NCC)]
            wo01_sb = wo_sb[:, 0, :]
            wo2_sb = wo_sb[:, 1, :]
            # dummy 1-elem exp: forces the ACT table load during the DMA
            # wait instead of on the first real exp (saves ~1.3us)
            dummy = consts.tile([1, 2], F32, name="dummy")
            nc.vector.memset(dummy[:], 0.0)
            nc.scalar.activation(dummy[:], dummy[:],
                                 mybir.ActivationFunctionType.Exp)
            # PE warm-up fodder (see pp pool below): zeros tile
            wu = consts.tile([128, 512], BF16, name="wu")
            nc.vector.memset(wu[:], 0.0)
            # prologue: 3 DMA rings; x split by TOKEN range so attention can
            # start after only tokens 0:512 (a1a+a1b) land.  Later x ranges
            # are chained BEHIND a1 (add_dep) so the critical prologue set
            # (wqk + a1) gets the full HBM bandwidth.
            #   scalar(HWDGE): a1a = x[c0:c3, 0:512],  a3 = x[:, 1024:1536]
            #   sync  (HWDGE): wqk, a1b = x[c3:c6, 0:512], a2, a4
            #   gpsimd(SWDGE): bqc, wv, wqk2, wo
            h_a1a = nc.scalar.dma_start(out=xT_sb[:, 0:3, 0:512],
                                        in_=xt_pcs[:, 0:3, 0:512])
            nc.sync.dma_start(out=wqk_sb[:], in_=wqk[:])
            nc.gpsimd.dma_start(out=bq_sb[:], in_=bqc[:])
            h_a1b = nc.sync.dma_start(out=xT_sb[:, 3:6, 0:512],
                                      in_=xt_pcs[:, 3:6, 0:512])
            nc.gpsimd.dma_start(out=wv_sb[:], in_=wv[:])
            h_a2 = nc.sync.dma_start(out=xT_sb[:, :, 512:1024],
                                     in_=xt_pcs[:, :, 512:1024])
            h_a3 = nc.scalar.dma_start(out=xT_sb[:, :, 1024:1536],
                                       in_=xt_pcs[:, :, 1024:1536])
            nc.sync.dma_start(out=xT_sb[:, :, 1536:2048],
                              in_=xt_pcs[:, :, 1536:2048])
            h_wqk2 = nc.gpsimd.dma_start(out=wqk2_sb[:], in_=wqk2[:])
            nc.gpsimd.dma_start(out=wo_sb[:], in_=wo[:])
            add_dep_helper(h_a2.ins, h_a1a.ins,
                           reason="x range 2 waits for critical range 1")
            add_dep_helper(h_a3.ins, h_a1b.ins,
                           reason="x range 3 waits for critical range 1")
            add_dep_helper(h_wqk2.ins, h_a2.ins,
                           reason="wqk2 not needed until pass 1")

            # heads 0,1 combined (h0 rows 0-63, h1 rows 64-127)
            qT01 = consts.tile([128, S], BF16, name="qT01")
            kT01 = consts.tile([128, S], BF16, name="kT01")
            # head 2, rows duplicated for row-group pairing
            qT2 = consts.tile([128, S], BF16, name="qT2")
            kT2 = consts.tile([128, S], BF16, name="kT2")
            vS = [consts.tile([128, HPG, HD + 1], BF16, name=f"vS{t}")
                  for t in range(NKB)]
            # PRE-SCALED attention outputs (h0 rows 0-63, h1 rows 64-127)
            oTs01 = [consts.tile([128, QH], BF16, name=f"oTs01_{qh}")
                     for qh in range(2)]
            oTs2 = [consts.tile([128, QH], BF16, name=f"oTs2_{qh}")
                    for qh in range(2)]
            s_bf = [consts.tile([1, S], F32, name=f"s_bf{h}")
                    for h in range(HPG)]
            # bf16 DMA staging for all 16 blocks
            ysb = [consts.tile([128, C], BF16, name=f"ysb{t}")
                   for t in range(NQB)]

            def qk_piece_body(pool, tag, w_list, n, kind):
                ps = pool.tile([128, 512], F32, tag=tag, name=f"{kind}{n}")
                for c in range(NCC):
                    nc.tensor.matmul(
                        ps[:],
                        w_list[c],
                        xT[c][:, n * 512 : (n + 1) * 512],
                        start=(c == 0),
                        stop=(c == NCC - 1),
                    )
                ns = slice(n * 512, (n + 1) * 512)
                if kind == "q":      # one combined copy + bias
                    nc.vector.tensor_scalar_add(
                        qT01[:, ns], ps[:], bq_sb[:, 0:1]
                    )
                elif kind == "k":
                    nc.vector.tensor_copy(kT01[:, ns], ps[:])
                else:  # 'qk2': q2 rows 0-63 (+bias, dup), k2 rows 64-127 (dup)
                    nc.vector.tensor_scalar_add(
                        qT2[0:64, ns], ps[0:64, :], bq_sb[0:64, 1:2]
                    )
                    nc.vector.tensor_scalar_add(
                        qT2[64:128, ns], ps[0:64, :], bq_sb[64:128, 1:2]
                    )
                    nc.vector.tensor_copy(kT2[0:64, ns], ps[64:128, :])
                    nc.vector.tensor_copy(kT2[64:128, ns], ps[64:128, :])

            def v_piece_body(pool, tag, t):
                vps = pool.tile([128, GF], F32, tag=tag, name=f"vp{t}")
                for c in range(NCC):
                    nc.tensor.matmul(
                        vps[:],
                        xT[c][:, t * 128 : (t + 1) * 128],
                        wv_sb[:, c, :],
                        start=(c == 0),
                        stop=(c == NCC - 1),
                    )
                nc.vector.tensor_copy(
                    vS[t][:, :, 0:HD],
                    vps[:].rearrange("p (h d) -> p h d", h=HPG),
                )
                nc.vector.memset(vS[t][:, :, HD : HD + 1], 1.0)

            # ------------- phase A: just enough to start attention -------------
            # v0/v1 are NOT computed here: they run as pre-drips of step
            # (0,0) so phase A only holds 2 PSUM banks and the attention
            # pools (ltp 4 + opp 2 + flex 2) can open without waiting on
            # serialized DVE copies.
            with tc.tile_pool(name="pp", bufs=2, space="PSUM") as pp:
                # q0/k0 interleaved by contraction chunk: the PE pipelines
                # across the xT chunk arrivals instead of stalling per piece
                ps_q = pp.tile([128, 512], F32, tag="pp", name="q0")
                ps_k = pp.tile([128, 512], F32, tag="pp", name="k0")
                # PE warm-up: dummy matmuls on zeros into ps_q (overwritten
                # by q0's start=True) while the x DMA is in flight, so the
                # PE p-state is at full clock when phase A starts
                for _ in range(16):
                    nc.tensor.matmul(ps_q[:], wu[:, 0:128], wu[:],
                                     start=True, stop=True)
                for c in range(NCC):
                    nc.tensor.matmul(ps_q[:], wq_c[c], xT[c][:, 0:512],
                                     start=(c == 0), stop=(c == NCC - 1))
                    nc.tensor.matmul(ps_k[:], wk_c[c], xT[c][:, 0:512],
                                     start=(c == 0), stop=(c == NCC - 1))
                nc.vector.tensor_scalar_add(qT01[:, 0:512], ps_q[:],
                                            bq_sb[:, 0:1])
                nc.vector.tensor_copy(kT01[:, 0:512], ps_k[:])

            # ---------------- attention: one pipelined stream ----------------
            with tc.tile_pool(name="flex", bufs=1, space="PSUM") as flex, \
                 tc.tile_pool(name="lt_ps", bufs=2, space="PSUM") as ltp, \
                 tc.tile_pool(name="o_ps", bufs=2, space="PSUM") as opp, \
                 tc.tile_pool(name="att_sb", bufs=4) as asb:

                def task_v(t):
                    def run():
                        v_piece_body(flex, "flex", t)
                    return run

                def task_qk(w_list, n, kind):
                    def run():
                        qk_piece_body(flex, "flex", w_list, n, kind)
                    return run

                ext_oraw = {}

                # deferred part of the softmax-scale chain (dripped into the
                # NEXT pass so the broadcast never stalls the in-order queues)
                def ext_finish(h, qa):
                    def run():
                        sb = asb.tile([HD, 512], F32, tag="sbc",
                                      name=f"sb{h}_{qa}")
                        nc.gpsimd.partition_broadcast(
                            sb[:], s_bf[h][:, qa : qa + 512], channels=HD
                        )
                        rb = asb.tile([HD, 512], F32, tag="rb",
                                      name=f"rb{h}_{qa}")
                        nc.vector.reciprocal_approx_fast(out=rb[:], in_=sb[:])
                        oraw = ext_oraw.pop((h, qa))
                        cs = slice(qa % QH, (qa % QH) + 512)
                        if h < 2:
                            dsts = [oTs01[qa // QH][h * 64 : (h + 1) * 64, cs]]
                        else:
                            dsts = [oTs2[qa // QH][0:64, cs],
                                    oTs2[qa // QH][64:128, cs]]
                        for dst in dsts:
                            nc.vector.tensor_tensor(out=dst, in0=oraw[:],
                                                    in1=rb[:], op=MULT)
                    return run

                def extract(o_ps, h, qa):
                    """Immediate extraction: frees o_ps (s row + raw O').
                    High priority: these copies gate the o-psum slots the
                    next pass's first oV matmuls need."""
                    with tc.high_priority():
                        nc.vector.tensor_copy(s_bf[h][:, qa : qa + 512],
                                              o_ps[HD : HD + 1, :])
                        oraw = asb.tile([HD, 512], BF16, tag="oraw",
                                        name=f"oraw{h}_{qa}")
                        nc.vector.tensor_copy(oraw[:], o_ps[0:HD, :])
                    ext_oraw[(h, qa)] = oraw

                def task_y_full(qh, t):
                    """single-shot Y for a token block: all 3 heads."""
                    def run():
                        yp = flex.tile([128, C], F32, tag="flex",
                                       name=f"y{qh}_{t}")
                        tbs = slice((t % 8) * 128, (t % 8) * 128 + 128)
                        nc.tensor.matmul(yp[:, 0:512], oTs01[qh][:, tbs],
                                         wo01_sb[:, 0:512],
                                         start=True, stop=False)
                        nc.tensor.matmul(yp[:, 512:C], oTs01[qh][:, tbs],
                                         wo01_sb[:, 512:C],
                                         start=True, stop=False)
                        nc.tensor.matmul(yp[:, 0:512], oTs2[qh][0:64, tbs],
                                         wo2_sb[0:64, 0:512],
                                         start=False, stop=True)
                        nc.tensor.matmul(yp[:, 512:C], oTs2[qh][64:128, tbs],
                                         wo2_sb[64:128, 512:C],
                                         start=False, stop=True)
                        nc.vector.tensor_copy(ysb[t][:], yp[:])
                        # sync queue only: the scalar engine is busy with exp
                        nc.sync.dma_start(out=out[t * 128 : (t + 1) * 128, :],
                                          in_=ysb[t][:])
                    return run

                # pass specs ------------------------------------------------
                def p01(qb):
                    qa = qb * 512
                    return (kT01, qT01, qa, kT01, qT01, qa,
                            (0, qa), (1, qa))

                def h2(qh):
                    qa = qh * QH
                    return (kT2, qT2, qa, kT2, qT2, qa + 512,
                            (2, qa), (2, qa + 512))

                passes = [p01(0), p01(1), h2(0), p01(2), h2(1), p01(3)]
                extra = {
                    0: [(0, task_v(2)), (0, task_v(3)), (1, task_v(4)),
                        (2, task_qk(wk_c, 1, "k")), (3, task_v(5)),
                        (4, task_v(6)), (5, task_qk(wk_c, 2, "k")),
                        (5, task_v(7)), (6, task_v(8)), (7, task_v(9)),
                        (8, task_qk(wk_c, 3, "k")), (8, task_v(10)),
                        (9, task_v(11)), (10, task_v(12)), (11, task_v(13)),
                        (12, task_v(14)), (13, task_v(15)),
                        (14, task_qk(wq_c, 1, "q"))],
                    # qk2 pieces 0+1 EARLY: h2(0)'s very first logits
                    # pair reads qT2 piece 0 (rows 0:64) AND piece 1 (rows
                    # 64:128) — both must clear the DVE dup copies well
                    # before the pass-1 -> pass-2 boundary
                    1: [(2, task_qk(wqk2_c, 0, "qk2")),
                        (5, task_qk(wqk2_c, 1, "qk2")),
                        (8, task_qk(wq_c, 2, "q"))],
                    2: [(2, task_qk(wqk2_c, 2, "qk2")),
                        (4, task_qk(wq_c, 3, "q")),
                        (6, task_qk(wqk2_c, 3, "qk2"))],
                    # p01(2): first-half Y blocks 0-3 (kb 0 kept drip-free:
                    # transitions are the tightest PE windows)
                    3: [(1, task_y_full(0, 0)), (4, task_y_full(0, 1)),
                        (7, task_y_full(0, 2)), (10, task_y_full(0, 3))],
                    # h2(1): first-half Y blocks 4-7
                    4: [(1, task_y_full(0, 4)), (4, task_y_full(0, 5)),
                        (7, task_y_full(0, 6)), (10, task_y_full(0, 7))],
                    # p01(3): blocks 8-11 single-shot, early so the pass's
                    # final kbs are drip-free ahead of the tail chain
                    5: [(1, task_y_full(1, 8)), (4, task_y_full(1, 9)),
                        (7, task_y_full(1, 10)), (10, task_y_full(1, 11))],
                }
                drips = {}
                for i, lst in extra.items():
                    for kb, ts in lst:
                        drips.setdefault((i, kb), []).append(ts)

                steps = [(i, kb) for i in range(len(passes))
                         for kb in range(NKB)]

                def emit_lt(i, kb):
                    (klo, qlo_t, qlo, khi, qhi_t, qhi, uA, uB) = passes[i]
                    kbs = slice(kb * 128, (kb + 1) * 128)
                    lt = ltp.tile([128, 1024], F32, tag="lt")
                    # high priority: the scheduler must keep the row-group
                    # pair adjacent (they co-execute on disjoint PE rows)
                    # and never insert drip matmuls before them — exp(s+1)
                    # is gated on this pair.  Priorities must be UNIQUE and
                    # ordered per step: with a constant (0,1) per pair, two
                    # simultaneously-ready pairs pop as (h0,h0,h64,h64) —
                    # same-row-group halves serialize on the PE.
                    po = tc.cur_priority
                    tc.cur_priority = 2 * (i * NKB + kb)
                    nc.tensor.matmul(
                        lt[:, 0:512], klo[0:64, kbs],
                        qlo_t[0:64, qlo : qlo + 512],
                        start=True, stop=True,
                    )
                    nc.tensor.matmul(
                        lt[:, 512:1024], khi[64:128, kbs],
                        qhi_t[64:128, qhi : qhi + 512],
                        start=True, stop=True,
                    )
                    tc.cur_priority = po
                    return lt

                o_cur = {}
                lt_cur = emit_lt(*steps[0])
                for s, (i, kb) in enumerate(steps):
                    uA, uB = passes[i][6], passes[i][7]
                    elt = asb.tile([128, 1024], BF16, tag="elt")
                    nc.scalar.activation(
                        elt[:], lt_cur[:], mybir.ActivationFunctionType.Exp
                    )
                    # software pipeline: next step's logits before this
                    # step's oV matmuls (keeps Act back-to-back)
                    if s + 1 < len(steps):
                        lt_cur = emit_lt(*steps[s + 1])
                    if s == 0:
                        # v0/v1 must be on the PE queue BEFORE the first oV
                        # matmuls (in-order queue: oV(0,0) waits on vS[0])
                        v_piece_body(flex, "flex", 0)
                        v_piece_body(flex, "flex", 1)
                    if kb == 0:
                        o_cur[0] = opp.tile([HD + 1, 512], F32, tag="o",
                                            name=f"oA{uA[0]}_{uA[1]}")
                        o_cur[1] = opp.tile([HD + 1, 512], F32, tag="o",
                                            name=f"oB{uB[0]}_{uB[1]}")
                    nc.tensor.matmul(
                        o_cur[0][:], vS[kb][:, uA[0], :], elt[:, 0:512],
                        start=(kb == 0), stop=(kb == NKB - 1),
                    )
                    nc.tensor.matmul(
                        o_cur[1][:], vS[kb][:, uB[0], :], elt[:, 512:1024],
                        start=(kb == 0), stop=(kb == NKB - 1),
                    )
                    for tsk in drips.get((i, kb), ()):
                        tsk()
                    if kb == NKB - 1:
                        last = i == len(passes) - 1
                        if not last:
                            extract(o_cur[0], *uA)
                            extract(o_cur[1], *uB)
                        else:
                            # last pass: s rows via DVE (hp), oraw copies
                            # via the now-idle scalar engine
                            for oc, u in ((o_cur[0], uA), (o_cur[1], uB)):
                                with tc.high_priority():
                                    nc.vector.tensor_copy(
                                        s_bf[u[0]][:, u[1] : u[1] + 512],
                                        oc[HD : HD + 1, :])
                                    orw = asb.tile([HD, 512], BF16,
                                                   tag="oraw",
                                                   name=f"orw{u[0]}_{u[1]}")
                                    nc.scalar.copy(orw[:], oc[0:HD, :])
                                ext_oraw[u] = orw
                        if last:
                            # pre-start the oTs2 half of tail-Y blocks
                            # 12-14 NOW: keeps the PE busy (and its p-state
                            # up) during the final softmax-scale chain
                            yps_tail = {}
                            for t, (pool_t, tg) in zip(
                                (12, 13, 14),
                                ((flex, "flex"), (ltp, "lt"), (ltp, "lt")),
                            ):
                                tbs = slice((t - 8) * 128,
                                            (t - 8) * 128 + 128)
                                yp = pool_t.tile([128, C], F32, tag=tg,
                                                 name=f"yt_{t}")
                                nc.tensor.matmul(
                                    yp[:, 0:512], oTs2[1][0:64, tbs],
                                    wo2_sb[0:64, 0:512],
                                    start=True, stop=False)
                                nc.tensor.matmul(
                                    yp[:, 512:C], oTs2[1][64:128, tbs],
                                    wo2_sb[64:128, 512:C],
                                    start=True, stop=False)
                                yps_tail[t] = yp
                        ext_finish(*uA)()
                        ext_finish(*uB)()

                # tail: blocks 12-15 (need oTs01[1] 2nd half from last pass)
                for t in range(12, 16):
                    tbs = slice((t - 8) * 128, (t - 8) * 128 + 128)
                    if t in yps_tail:
                        yp = yps_tail[t]
                        nc.tensor.matmul(yp[:, 0:512], oTs01[1][:, tbs],
                                         wo01_sb[:, 0:512],
                                         start=False, stop=True)
                        nc.tensor.matmul(yp[:, 512:C], oTs01[1][:, tbs],
                                         wo01_sb[:, 512:C],
                                         start=False, stop=True)
                    else:
                        yp = flex.tile([128, C], F32, tag="flex",
                                       name=f"yt_{t}")
                        nc.tensor.matmul(yp[:, 0:512], oTs01[1][:, tbs],
                                         wo01_sb[:, 0:512],
                                         start=True, stop=False)
                        nc.tensor.matmul(yp[:, 512:C], oTs01[1][:, tbs],
                                         wo01_sb[:, 512:C],
                                         start=True, stop=False)
                        nc.tensor.matmul(yp[:, 0:512], oTs2[1][0:64, tbs],
                                         wo2_sb[0:64, 0:512],
                                         start=False, stop=True)
                        nc.tensor.matmul(yp[:, 512:C], oTs2[1][64:128, tbs],
                                         wo2_sb[64:128, 512:C],
                                         start=False, stop=True)
                    # split the psum->bf16 casts across DVE and the (now
                    # idle) scalar engine so the tail doesn't serialize
                    if t % 2 == 0:
                        nc.scalar.copy(ysb[t][:], yp[:])
                    else:
                        nc.vector.tensor_copy(ysb[t][:], yp[:])
                    eng = nc.sync if t % 2 == 0 else nc.scalar
                    eng.dma_start(out=out[t * 128 : (t + 1) * 128, :],
                                  in_=ysb[t][:])

    nc.compile()
    return nc


_COMPILED_NC = None


def _get_nc():
    global _COMPILED_NC
    if _COMPILED_NC is None:
        _COMPILED_NC = build_program()
    return _COMPILED_NC


def _pack_chunks(w):
    # [768, F] -> [128, NCC, F]: partition p, chunk c <- row c*128+p
    f = w.shape[1]
    return np.ascontiguousarray(
        w.reshape(NCC, 128, f).transpose(1, 0, 2)
    )


def make_in_maps(x, Wq, bq, Wk, bk, Wv, bv, Wo, bo):
    scale = 1.0 / np.sqrt(HD)
    bf = ml_dtypes.bfloat16
    # host-side pre-transpose: kernel takes x^T [C, S]
    x_bf = [np.ascontiguousarray(x[b].T).astype(bf) for b in range(x.shape[0])]
    Wq = np.asarray(Wq)
    Wk = np.asarray(Wk)
    Wv = np.asarray(Wv)
    Wo = np.asarray(Wo)
    bq = np.asarray(bq)
    in_maps = []
    for cid in range(NCORES):
        b, g = divmod(cid, GROUPS)
        cols = slice(g * GF, (g + 1) * GF)
        wq_g = Wq[:, cols] * scale
        wk_g = Wk[:, cols]
        wqk2 = np.concatenate([wq_g[:, 128:192], wk_g[:, 128:192]], axis=1)
        wqk_h = np.concatenate([wq_g[:, 0:128], wk_g[:, 0:128]], axis=1)
        bq_g = bq[cols] * scale
        bqc = np.empty((128, 2), dtype=np.float32)
        bqc[:, 0] = bq_g[0:128]            # [bq_h0 | bq_h1]
        bqc[0:64, 1] = bq_g[128:192]       # bq_h2 duplicated
        bqc[64:128, 1] = bq_g[128:192]
        wo_g = Wo[cols, :]
        wo_h = np.stack(
            [wo_g[0:128, :],
             np.concatenate([wo_g[128:192, :]] * 2, axis=0)], axis=1
        )  # [128, 2, 768]
        in_maps.append(
            {
                "x": x_bf[b],
                "wqk": _pack_chunks(wqk_h).astype(bf),
                "wqk2": _pack_chunks(wqk2).astype(bf),
                "wv": _pack_chunks(Wv[:, cols]).astype(bf),
                "wo": np.ascontiguousarray(wo_h).astype(bf),
                "bqc": bqc,
            }
        )
    return in_maps


def gather_output(results, x, Wv, bv, Wo, bo):
    B = x.shape[0]
    out = np.zeros((B, S, C), dtype=np.float32)
    for cid in range(NCORES):
        b, _ = divmod(cid, GROUPS)
        out[b] += results[cid]["out"].astype(np.float32)
    # exact bias folds: bk cancels in softmax; v-bias -> bv @ Wo; + bo
    out += (np.asarray(bv, np.float32) @ np.asarray(Wo, np.float32)
            + np.asarray(bo, np.float32))
    return out


def kernel(x, Wq, bq, Wk, bk, Wv, bv, Wo, bo):
    x = np.asarray(x)
    nc = _get_nc()
    in_maps = make_in_maps(x, Wq, bq, Wk, bk, Wv, bv, Wo, bo)
    res = run_bass_kernel_spmd(nc, in_maps, core_ids=list(range(NCORES)))
    return gather_output(res.results, x, Wv, bv, Wo, bo)
